# revision 2
# baseline (speedup 1.0000x reference)
"""Trainium2 Bass kernel for the dense_mlp problem (8 NeuronCores, data parallel).

Network: x[N,2] -> Linear(2,16)+tanh -> 8x (Linear(16,16)+tanh)
         -> Linear(16,3)+sigmoid, N = 4_194_304.

Strategy (per core, N_shard = 524288 pixels):
  - 8 pixel-streams of 65536 pixels; activations kept transposed in SBUF
    as [128, cols] tiles (partition 16*s+j = feature j of stream s).
  - Weights block-diagonal [128,128] (8 copies of W.T), so each matmul
    column advances 8 pixels. Matmuls run in float32r (TF32) at
    1 cycle/row; PSUM accumulates fp32.
  - tanh/sigmoid fused into single ACTIVATE ops over [128, 2048] PSUM
    spans (ScalarE is the bottleneck engine: ~77M activation evals/core).
  - Host does the free layout transposes (shard / interleave).
"""

import numpy as np

import concourse.bass as bass
import concourse.mybir as mybir
from concourse import bacc
from concourse.bass_utils import run_bass_kernel_spmd
from concourse.tile import TileContext

N_HID_LAYERS = 8
N_STREAMS = 8
N_CORES = 8
N_COLS = 65536          # pixels per stream per core
CHUNK = 2048
MM_BLOCK = 512

AF = mybir.ActivationFunctionType

LAST_RUN_INFO = {}

_GRAPH_CACHE = {}


def _build_graph(n_cols, chunk=CHUNK, mm_block=MM_BLOCK):
    key = (n_cols, chunk, mm_block)
    if key in _GRAPH_CACHE:
        return _GRAPH_CACHE[key]
    n_chunks = n_cols // chunk
    blocks = chunk // mm_block

    nc = bacc.Bacc()
    f32 = mybir.dt.float32
    mm_dt = mybir.dt.float32r

    xT = nc.declare_dram_parameter("xT", [2 * N_STREAMS, n_cols], mm_dt, isOutput=False)
    w0 = nc.declare_dram_parameter("w0", [2 * N_STREAMS, 128], mm_dt, isOutput=False)
    b0 = nc.declare_dram_parameter("b0", [128, 1], f32, isOutput=False)
    wh = nc.declare_dram_parameter("wh", [N_HID_LAYERS, 128, 128], mm_dt, isOutput=False)
    wo = nc.declare_dram_parameter("wo", [128, 3 * N_STREAMS], mm_dt, isOutput=False)
    out = nc.declare_dram_parameter("out", [3 * N_STREAMS, n_cols], f32, isOutput=True)

    with TileContext(nc) as tc:
        with (
            tc.tile_pool(name="wpool", bufs=1) as wpool,
            tc.tile_pool(name="xpool", bufs=3) as xpool,
            tc.tile_pool(name="hpool", bufs=4) as hpool,
            tc.tile_pool(name="opool", bufs=3) as opool,
            tc.tile_pool(name="psum", bufs=2, space="PSUM") as psum_pool,
        ):
            w0_sb = wpool.tile([2 * N_STREAMS, 128], mm_dt)
            nc.sync.dma_start(out=w0_sb, in_=w0[:, :])
            b0_sb = wpool.tile([128, 1], f32)
            nc.sync.dma_start(out=b0_sb, in_=b0[:, :])
            wh_sb = []
            for l in range(N_HID_LAYERS):
                w_l = wpool.tile([128, 128], mm_dt, name=f"wh{l}")
                nc.sync.dma_start(out=w_l, in_=wh[l, :, :])
                wh_sb.append(w_l)
            wo_sb = wpool.tile([128, 3 * N_STREAMS], mm_dt)
            nc.sync.dma_start(out=wo_sb, in_=wo[:, :])

            for c in range(n_chunks):
                cs = slice(c * chunk, (c + 1) * chunk)
                x_sb = xpool.tile([2 * N_STREAMS, chunk], mm_dt, tag="x")
                nc.sync.dma_start(out=x_sb, in_=xT[:, cs])

                ps = psum_pool.tile([128, chunk], f32, tag="ps")
                for k in range(blocks):
                    bs = slice(k * mm_block, (k + 1) * mm_block)
                    nc.tensor.matmul(ps[:, bs], w0_sb, x_sb[:, bs],
                                     start=True, stop=True)
                h = hpool.tile([128, chunk], mm_dt, tag="h")
                nc.scalar.activation(h, ps, AF.Tanh, bias=b0_sb)

                for l in range(N_HID_LAYERS):
                    ps = psum_pool.tile([128, chunk], f32, tag="ps")
                    for k in range(blocks):
                        bs = slice(k * mm_block, (k + 1) * mm_block)
                        nc.tensor.matmul(ps[:, bs], wh_sb[l],
                                         h[:, bs], start=True, stop=True)
                    h_next = hpool.tile([128, chunk], mm_dt, tag="h")
                    nc.scalar.activation(h_next, ps, AF.Tanh)
                    h = h_next

                ps = psum_pool.tile([128, chunk], f32, tag="ps")
                for k in range(blocks):
                    bs = slice(k * mm_block, (k + 1) * mm_block)
                    nc.tensor.matmul(ps[:3 * N_STREAMS, bs], wo_sb,
                                     h[:, bs], start=True, stop=True)
                o_sb = opool.tile([3 * N_STREAMS, chunk], f32, tag="o")
                nc.scalar.activation(o_sb, ps[:3 * N_STREAMS, :], AF.Sigmoid)
                nc.sync.dma_start(out=out[:, cs], in_=o_sb)

    nc.compile()
    _GRAPH_CACHE[key] = nc
    return nc


def _round_tf32(a):
    """Round float32 to TF32 (FP32r): round-to-nearest to 10 mantissa bits."""
    u = np.ascontiguousarray(a, np.float32).view(np.uint32)
    r = (u + 0x0FFF + ((u >> 13) & 1)) & np.uint32(0xFFFFE000)
    return r.view(np.float32)


def _pack_weights(W0, b0, Wh, Wout):
    w0p = np.zeros((2 * N_STREAMS, 128), np.float32)
    b0p = np.zeros((128, 1), np.float32)
    whp = np.zeros((N_HID_LAYERS, 128, 128), np.float32)
    wop = np.zeros((128, 3 * N_STREAMS), np.float32)
    for s in range(N_STREAMS):
        w0p[2 * s:2 * s + 2, 16 * s:16 * s + 16] = W0.T
        b0p[16 * s:16 * s + 16, 0] = b0
        for l in range(N_HID_LAYERS):
            whp[l, 16 * s:16 * s + 16, 16 * s:16 * s + 16] = Wh[l].T
        wop[16 * s:16 * s + 16, 3 * s:3 * s + 3] = Wout.T
    return w0p, b0p, whp, wop


def kernel(x, W0, b0, Wh, Wout, trace=False):
    x = np.asarray(x, np.float32)
    W0 = np.asarray(W0, np.float32)
    b0 = np.asarray(b0, np.float32)
    Wh = np.asarray(Wh, np.float32)
    Wout = np.asarray(Wout, np.float32)

    nc = _build_graph(N_COLS)
    w0p, b0p, whp, wop = _pack_weights(W0, b0, Wh, Wout)
    w0p, whp, wop = _round_tf32(w0p), _round_tf32(whp), _round_tf32(wop)

    per_core = N_STREAMS * N_COLS
    in_maps = []
    for c in range(N_CORES):
        xs = x[c * per_core:(c + 1) * per_core]
        xT = np.ascontiguousarray(
            xs.reshape(N_STREAMS, N_COLS, 2).transpose(0, 2, 1)
        ).reshape(2 * N_STREAMS, N_COLS)
        in_maps.append({"xT": _round_tf32(xT), "w0": w0p, "b0": b0p,
                        "wh": whp, "wo": wop})

    res = run_bass_kernel_spmd(nc, in_maps, core_ids=list(range(N_CORES)),
                               trace=trace)
    LAST_RUN_INFO.clear()
    LAST_RUN_INFO["exec_time_ns"] = res.exec_time_ns
    prof = getattr(res, "instructions_and_trace", None)
    LAST_RUN_INFO["trace_dir"] = getattr(prof, "trace_dir", None)

    parts = []
    for r in res.results:
        o = r["out"].reshape(N_STREAMS, 3, N_COLS).transpose(0, 2, 1)
        parts.append(o.reshape(per_core, 3))
    return np.concatenate(parts, axis=0)


# revision 3
# speedup vs baseline: 2.0589x; 2.0589x over previous
"""Trainium2 Bass kernel for the dense_mlp problem (8 NeuronCores, data parallel).

Network: x[N,2] -> Linear(2,16)+tanh -> 8x (Linear(16,16)+tanh)
         -> Linear(16,3)+sigmoid, N = 4_194_304.

Strategy (per core, N_shard = 524288 pixels):
  - 8 pixel-streams of 65536 pixels; activations kept transposed in SBUF
    as [128, cols] tiles (partition 16*s+j = feature j of stream s).
  - Weights block-diagonal [128,128] (8 copies of W.T), so each matmul
    column advances 8 pixels. Matmuls in bf16 (1 cycle/row, fast weight
    load); PSUM accumulates fp32; tanh applied in fp32 out of PSUM.
  - tanh/sigmoid fused into single ACTIVATE ops over [128, 2048] PSUM
    spans (ScalarE is the bottleneck engine: ~77M activation evals/core).
  - Chunks processed in interleaved pairs so TensorE matmuls of one
    chunk overlap ScalarE activations of the other.
  - Host does the free layout transposes (shard / interleave).
"""

import ml_dtypes
import numpy as np

import concourse.bass as bass
import concourse.mybir as mybir
from concourse import bacc
from concourse.bass_utils import run_bass_kernel_spmd
from concourse.tile import TileContext

N_HID_LAYERS = 8
N_STREAMS = 8
N_CORES = 8
N_COLS = 65536          # pixels per stream per core
CHUNK = 2048
MM_BLOCK = 512

AF = mybir.ActivationFunctionType
BF16 = ml_dtypes.bfloat16

LAST_RUN_INFO = {}

_GRAPH_CACHE = {}


def _build_graph(n_cols, chunk=CHUNK, mm_block=MM_BLOCK):
    key = (n_cols, chunk, mm_block)
    if key in _GRAPH_CACHE:
        return _GRAPH_CACHE[key]
    n_pairs = n_cols // (2 * chunk)
    blocks = chunk // mm_block

    nc = bacc.Bacc()
    f32 = mybir.dt.float32
    mm_dt = mybir.dt.bfloat16

    xT = nc.declare_dram_parameter("xT", [2 * N_STREAMS, n_cols], mm_dt, isOutput=False)
    w0 = nc.declare_dram_parameter("w0", [2 * N_STREAMS, 128], mm_dt, isOutput=False)
    b0 = nc.declare_dram_parameter("b0", [128, 1], f32, isOutput=False)
    wh = nc.declare_dram_parameter("wh", [N_HID_LAYERS, 128, 128], mm_dt, isOutput=False)
    wo = nc.declare_dram_parameter("wo", [128, 3 * N_STREAMS], mm_dt, isOutput=False)
    out = nc.declare_dram_parameter("out", [3 * N_STREAMS, n_cols], f32, isOutput=True)

    with TileContext(nc) as tc:
        with (
            tc.tile_pool(name="wpool", bufs=1) as wpool,
            tc.tile_pool(name="xpool", bufs=4) as xpool,
            tc.tile_pool(name="hpool", bufs=6) as hpool,
            tc.tile_pool(name="opool", bufs=4) as opool,
            tc.tile_pool(name="psum", bufs=2, space="PSUM") as psum_pool,
        ):
            w0_sb = wpool.tile([2 * N_STREAMS, 128], mm_dt)
            nc.sync.dma_start(out=w0_sb, in_=w0[:, :])
            b0_sb = wpool.tile([128, 1], f32)
            nc.sync.dma_start(out=b0_sb, in_=b0[:, :])
            wh_sb = []
            for l in range(N_HID_LAYERS):
                w_l = wpool.tile([128, 128], mm_dt, name=f"wh{l}")
                nc.sync.dma_start(out=w_l, in_=wh[l, :, :])
                wh_sb.append(w_l)
            wo_sb = wpool.tile([128, 3 * N_STREAMS], mm_dt)
            nc.sync.dma_start(out=wo_sb, in_=wo[:, :])

            def mm_layer(ps, w_sb, rhs, m=None):
                for k in range(blocks):
                    bs = slice(k * mm_block, (k + 1) * mm_block)
                    o = ps[:, bs] if m is None else ps[:m, bs]
                    nc.tensor.matmul(o, w_sb, rhs[:, bs], start=True, stop=True)

            for p in range(n_pairs):
                css = [slice((2 * p + i) * chunk, (2 * p + i + 1) * chunk)
                       for i in range(2)]
                x_sb, h, o_sb, ps = [None, None], [None, None], [None, None], [None, None]
                for i in range(2):
                    x_sb[i] = xpool.tile([2 * N_STREAMS, chunk], mm_dt, tag="x",
                                         name=f"x{i}")
                    nc.sync.dma_start(out=x_sb[i], in_=xT[:, css[i]])

                # layer 0
                for i in range(2):
                    ps[i] = psum_pool.tile([128, chunk], f32, tag="ps", name=f"ps{i}")
                    mm_layer(ps[i], w0_sb, x_sb[i])
                for i in range(2):
                    h[i] = hpool.tile([128, chunk], mm_dt, tag="h", name=f"h{i}")
                    nc.scalar.activation(h[i], ps[i], AF.Tanh, bias=b0_sb)

                # hidden layers
                for l in range(N_HID_LAYERS):
                    for i in range(2):
                        ps[i] = psum_pool.tile([128, chunk], f32, tag="ps",
                                               name=f"ps{i}")
                        mm_layer(ps[i], wh_sb[l], h[i])
                    for i in range(2):
                        h_next = hpool.tile([128, chunk], mm_dt, tag="h",
                                            name=f"h{i}")
                        nc.scalar.activation(h_next, ps[i], AF.Tanh)
                        h[i] = h_next

                # output layer + sigmoid + store
                for i in range(2):
                    ps[i] = psum_pool.tile([128, chunk], f32, tag="ps", name=f"ps{i}")
                    mm_layer(ps[i], wo_sb, h[i], m=3 * N_STREAMS)
                for i in range(2):
                    o_sb[i] = opool.tile([3 * N_STREAMS, chunk], f32, tag="o",
                                         name=f"o{i}")
                    nc.scalar.activation(o_sb[i], ps[i][:3 * N_STREAMS, :],
                                         AF.Sigmoid)
                    nc.sync.dma_start(out=out[:, css[i]], in_=o_sb[i])

    nc.compile()
    _GRAPH_CACHE[key] = nc
    return nc


def _pack_weights(W0, b0, Wh, Wout):
    w0p = np.zeros((2 * N_STREAMS, 128), np.float32)
    b0p = np.zeros((128, 1), np.float32)
    whp = np.zeros((N_HID_LAYERS, 128, 128), np.float32)
    wop = np.zeros((128, 3 * N_STREAMS), np.float32)
    for s in range(N_STREAMS):
        w0p[2 * s:2 * s + 2, 16 * s:16 * s + 16] = W0.T
        b0p[16 * s:16 * s + 16, 0] = b0
        for l in range(N_HID_LAYERS):
            whp[l, 16 * s:16 * s + 16, 16 * s:16 * s + 16] = Wh[l].T
        wop[16 * s:16 * s + 16, 3 * s:3 * s + 3] = Wout.T
    return w0p, b0p, whp, wop


def kernel(x, W0, b0, Wh, Wout, trace=False):
    x = np.asarray(x, np.float32)
    W0 = np.asarray(W0, np.float32)
    b0 = np.asarray(b0, np.float32)
    Wh = np.asarray(Wh, np.float32)
    Wout = np.asarray(Wout, np.float32)

    nc = _build_graph(N_COLS)
    w0p, b0p, whp, wop = _pack_weights(W0, b0, Wh, Wout)
    w0p, whp, wop = (a.astype(BF16) for a in (w0p, whp, wop))

    per_core = N_STREAMS * N_COLS
    in_maps = []
    for c in range(N_CORES):
        xs = x[c * per_core:(c + 1) * per_core]
        xT = np.ascontiguousarray(
            xs.reshape(N_STREAMS, N_COLS, 2).transpose(0, 2, 1)
        ).reshape(2 * N_STREAMS, N_COLS)
        in_maps.append({"xT": xT.astype(BF16), "w0": w0p, "b0": b0p,
                        "wh": whp, "wo": wop})

    res = run_bass_kernel_spmd(nc, in_maps, core_ids=list(range(N_CORES)),
                               trace=trace)
    LAST_RUN_INFO.clear()
    LAST_RUN_INFO["exec_time_ns"] = res.exec_time_ns
    prof = getattr(res, "instructions_and_trace", None)
    LAST_RUN_INFO["trace_dir"] = getattr(prof, "trace_dir", None)

    parts = []
    for r in res.results:
        o = r["out"].reshape(N_STREAMS, 3, N_COLS).transpose(0, 2, 1)
        parts.append(o.reshape(per_core, 3))
    return np.concatenate(parts, axis=0)


# revision 7
# speedup vs baseline: 2.1348x; 1.0369x over previous
"""Trainium2 Bass kernel for the dense_mlp problem (8 NeuronCores, data parallel).

Network: x[N,2] -> Linear(2,16)+tanh -> 8x (Linear(16,16)+tanh)
         -> Linear(16,3)+sigmoid, N = 4_194_304.

Strategy (per core, N_shard = 524288 pixels):
  - 8 pixel-streams of 65536 pixels; activations kept transposed in SBUF
    as [128, cols] tiles (partition 16*s+j = feature j of stream s).
  - Weights block-diagonal [128,128] (8 copies of W.T), so each matmul
    column advances 8 pixels. Matmuls in bf16 (1 cycle/row, fast weight
    load); PSUM accumulates fp32; tanh applied in fp32 out of PSUM.
  - tanh/sigmoid fused into single ACTIVATE ops over [128, 2048] PSUM
    spans (ScalarE is the bottleneck engine: ~77M activation evals/core).
  - Chunks processed in interleaved pairs so TensorE matmuls of one
    chunk overlap ScalarE activations of the other.
  - Host does the free layout transposes (shard / interleave).
"""

import ml_dtypes
import numpy as np

import concourse.bass as bass
import concourse.mybir as mybir
from concourse import bacc
from concourse.bass_utils import run_bass_kernel_spmd
from concourse.tile import TileContext

N_HID_LAYERS = 8
N_STREAMS = 8
N_CORES = 8
N_COLS = 65536          # pixels per stream per core
CHUNK = 2048
MM_BLOCK = 512

AF = mybir.ActivationFunctionType
BF16 = ml_dtypes.bfloat16

LAST_RUN_INFO = {}

_GRAPH_CACHE = {}


def _build_graph(n_cols, chunk=CHUNK, mm_block=MM_BLOCK):
    key = (n_cols, chunk, mm_block)
    if key in _GRAPH_CACHE:
        return _GRAPH_CACHE[key]
    n_pairs = n_cols // (2 * chunk)
    blocks = chunk // mm_block

    nc = bacc.Bacc()
    f32 = mybir.dt.float32
    mm_dt = mybir.dt.bfloat16

    xT = nc.declare_dram_parameter("xT", [2 * N_STREAMS, n_cols], mm_dt, isOutput=False)
    w0 = nc.declare_dram_parameter("w0", [2 * N_STREAMS, 128], mm_dt, isOutput=False)
    b0 = nc.declare_dram_parameter("b0", [128, 1], f32, isOutput=False)
    wh = nc.declare_dram_parameter("wh", [N_HID_LAYERS, 128, 128], mm_dt, isOutput=False)
    wo = nc.declare_dram_parameter("wo", [128, 3 * N_STREAMS], mm_dt, isOutput=False)
    out = nc.declare_dram_parameter("out", [3 * N_STREAMS, n_cols], f32, isOutput=True)

    with TileContext(nc) as tc:
        with (
            tc.tile_pool(name="wpool", bufs=1) as wpool,
            tc.tile_pool(name="xpool", bufs=4) as xpool,
            tc.tile_pool(name="hpool", bufs=6) as hpool,
            tc.tile_pool(name="opool", bufs=4) as opool,
            tc.tile_pool(name="psum", bufs=2, space="PSUM") as psum_pool,
        ):
            w0_sb = wpool.tile([2 * N_STREAMS, 128], mm_dt)
            nc.sync.dma_start(out=w0_sb, in_=w0[:, :])
            b0_sb = wpool.tile([128, 1], f32)
            nc.sync.dma_start(out=b0_sb, in_=b0[:, :])
            wh_sb = []
            for l in range(N_HID_LAYERS):
                w_l = wpool.tile([128, 128], mm_dt, name=f"wh{l}")
                nc.sync.dma_start(out=w_l, in_=wh[l, :, :])
                wh_sb.append(w_l)
            wo_sb = wpool.tile([128, 3 * N_STREAMS], mm_dt)
            nc.sync.dma_start(out=wo_sb, in_=wo[:, :])

            def mm_layer(ps, w_sb, rhs, m=None):
                for k in range(blocks):
                    bs = slice(k * mm_block, (k + 1) * mm_block)
                    o = ps[:, bs] if m is None else ps[:m, bs]
                    nc.tensor.matmul(o, w_sb, rhs[:, bs], start=True, stop=True)

            for p in range(n_pairs):
                css = [slice((2 * p + i) * chunk, (2 * p + i + 1) * chunk)
                       for i in range(2)]
                x_sb, h, o_sb, ps = [None, None], [None, None], [None, None], [None, None]
                for i in range(2):
                    x_sb[i] = xpool.tile([2 * N_STREAMS, chunk], mm_dt, tag="x",
                                         name=f"x{i}")
                    nc.sync.dma_start(out=x_sb[i], in_=xT[:, css[i]])

                # layer 0
                for i in range(2):
                    ps[i] = psum_pool.tile([128, chunk], f32, tag="ps", name=f"ps{i}")
                    mm_layer(ps[i], w0_sb, x_sb[i])
                for i in range(2):
                    h[i] = hpool.tile([128, chunk], mm_dt, tag="h", name=f"h{i}")
                    nc.scalar.activation(h[i], ps[i], AF.Tanh, bias=b0_sb)

                # hidden layers
                for l in range(N_HID_LAYERS):
                    for i in range(2):
                        ps[i] = psum_pool.tile([128, chunk], f32, tag="ps",
                                               name=f"ps{i}")
                        mm_layer(ps[i], wh_sb[l], h[i])
                    for i in range(2):
                        h_next = hpool.tile([128, chunk], mm_dt, tag="h",
                                            name=f"h{i}")
                        nc.scalar.activation(h_next, ps[i], AF.Tanh)
                        h[i] = h_next

                # output layer: stack the 4 col-blocks of each chunk onto 4
                # PSUM partition groups (32k..32k+24) so sigmoid runs dense
                # on [128, 1024] once per pair instead of 2x [24, 2048].
                ps_o = psum_pool.tile([128, chunk], f32, tag="ps", name="pso")
                for i in range(2):
                    for k in range(blocks):
                        bs = slice(k * mm_block, (k + 1) * mm_block)
                        os_ = slice(i * mm_block, (i + 1) * mm_block)
                        nc.tensor.matmul(
                            ps_o[32 * k:32 * k + 3 * N_STREAMS, os_], wo_sb,
                            h[i][:, bs], start=True, stop=True,
                            tile_position=(0, 32 * k))
                o_pair = opool.tile([128, 2 * mm_block], f32, tag="o", name="o")
                nc.scalar.activation(o_pair, ps_o[:, :2 * mm_block], AF.Sigmoid)
                # o_pair[32k+m, 512i+c] = out[m, (2p+i)*2048 + 512k + c]
                for k in range(blocks):
                    src = o_pair[32 * k:32 * k + 3 * N_STREAMS, :]
                    dst = out.rearrange("m (q i kk c) -> kk m q i c",
                                        q=n_pairs, i=2, kk=blocks)[k, :, p]
                    nc.sync.dma_start(out=dst, in_=src)

    nc.compile()
    _GRAPH_CACHE[key] = nc
    return nc


def _pack_weights(W0, b0, Wh, Wout):
    w0p = np.zeros((2 * N_STREAMS, 128), np.float32)
    b0p = np.zeros((128, 1), np.float32)
    whp = np.zeros((N_HID_LAYERS, 128, 128), np.float32)
    wop = np.zeros((128, 3 * N_STREAMS), np.float32)
    for s in range(N_STREAMS):
        w0p[2 * s:2 * s + 2, 16 * s:16 * s + 16] = W0.T
        b0p[16 * s:16 * s + 16, 0] = b0
        for l in range(N_HID_LAYERS):
            whp[l, 16 * s:16 * s + 16, 16 * s:16 * s + 16] = Wh[l].T
        wop[16 * s:16 * s + 16, 3 * s:3 * s + 3] = Wout.T
    return w0p, b0p, whp, wop


def kernel(x, W0, b0, Wh, Wout, trace=False):
    x = np.asarray(x, np.float32)
    W0 = np.asarray(W0, np.float32)
    b0 = np.asarray(b0, np.float32)
    Wh = np.asarray(Wh, np.float32)
    Wout = np.asarray(Wout, np.float32)

    nc = _build_graph(N_COLS)
    w0p, b0p, whp, wop = _pack_weights(W0, b0, Wh, Wout)
    w0p, whp, wop = (a.astype(BF16) for a in (w0p, whp, wop))

    per_core = N_STREAMS * N_COLS
    in_maps = []
    for c in range(N_CORES):
        xs = x[c * per_core:(c + 1) * per_core]
        xT = np.ascontiguousarray(
            xs.reshape(N_STREAMS, N_COLS, 2).transpose(0, 2, 1)
        ).reshape(2 * N_STREAMS, N_COLS)
        in_maps.append({"xT": xT.astype(BF16), "w0": w0p, "b0": b0p,
                        "wh": whp, "wo": wop})

    res = run_bass_kernel_spmd(nc, in_maps, core_ids=list(range(N_CORES)),
                               trace=trace)
    LAST_RUN_INFO.clear()
    LAST_RUN_INFO["exec_time_ns"] = res.exec_time_ns
    prof = getattr(res, "instructions_and_trace", None)
    LAST_RUN_INFO["trace_dir"] = getattr(prof, "trace_dir", None)

    parts = []
    for r in res.results:
        o = r["out"].reshape(N_STREAMS, 3, N_COLS).transpose(0, 2, 1)
        parts.append(o.reshape(per_core, 3))
    return np.concatenate(parts, axis=0)


# revision 8
# speedup vs baseline: 2.1839x; 1.0230x over previous
"""Trainium2 Bass kernel for the dense_mlp problem (8 NeuronCores, data parallel).

Network: x[N,2] -> Linear(2,16)+tanh -> 8x (Linear(16,16)+tanh)
         -> Linear(16,3)+sigmoid, N = 4_194_304.

Strategy (per core, N_shard = 524288 pixels):
  - 8 pixel-streams of 65536 pixels; activations kept transposed in SBUF
    as [128, cols] tiles (partition 16*s+j = feature j of stream s).
  - Weights block-diagonal [128,128] (8 copies of W.T), so each matmul
    column advances 8 pixels. Matmuls in bf16 (1 cycle/row, fast weight
    load); PSUM accumulates fp32; tanh applied in fp32 out of PSUM.
  - tanh/sigmoid fused into single ACTIVATE ops over [128, 2048] PSUM
    spans (ScalarE is the bottleneck engine: ~77M activation evals/core).
  - Chunks processed in interleaved pairs so TensorE matmuls of one
    chunk overlap ScalarE activations of the other.
  - Host does the free layout transposes (shard / interleave).
"""

import ml_dtypes
import numpy as np

import concourse.bass as bass
import concourse.mybir as mybir
from concourse import bacc
from concourse.bass_utils import run_bass_kernel_spmd
from concourse.tile import TileContext

N_HID_LAYERS = 8
N_STREAMS = 8
N_CORES = 8
N_COLS = 65536          # pixels per stream per core
CHUNK = 2048
MM_BLOCK = 512

AF = mybir.ActivationFunctionType
BF16 = ml_dtypes.bfloat16

LAST_RUN_INFO = {}

_GRAPH_CACHE = {}


def _build_graph(n_cols, chunk=CHUNK, mm_block=MM_BLOCK):
    key = (n_cols, chunk, mm_block)
    if key in _GRAPH_CACHE:
        return _GRAPH_CACHE[key]
    GRP = 4
    n_grp = n_cols // (GRP * chunk)
    blocks = chunk // mm_block

    nc = bacc.Bacc()
    f32 = mybir.dt.float32
    mm_dt = mybir.dt.bfloat16

    xT = nc.declare_dram_parameter("xT", [2 * N_STREAMS, n_cols], mm_dt, isOutput=False)
    w0 = nc.declare_dram_parameter("w0", [2 * N_STREAMS, 128], mm_dt, isOutput=False)
    b0 = nc.declare_dram_parameter("b0", [128, 1], f32, isOutput=False)
    wh = nc.declare_dram_parameter("wh", [N_HID_LAYERS, 128, 128], mm_dt, isOutput=False)
    wo = nc.declare_dram_parameter("wo", [128, 3 * N_STREAMS], mm_dt, isOutput=False)
    out = nc.declare_dram_parameter("out", [3 * N_STREAMS, n_cols], f32, isOutput=True)

    with TileContext(nc) as tc:
        with (
            tc.tile_pool(name="wpool", bufs=1) as wpool,
            tc.tile_pool(name="xpool", bufs=6) as xpool,
            tc.tile_pool(name="hpool", bufs=8) as hpool,
            tc.tile_pool(name="opool", bufs=4) as opool,
            tc.tile_pool(name="psum", bufs=2, space="PSUM") as psum_pool,
        ):
            w0_sb = wpool.tile([2 * N_STREAMS, 128], mm_dt)
            nc.sync.dma_start(out=w0_sb, in_=w0[:, :])
            b0_sb = wpool.tile([128, 1], f32)
            nc.sync.dma_start(out=b0_sb, in_=b0[:, :])
            wh_sb = []
            for l in range(N_HID_LAYERS):
                w_l = wpool.tile([128, 128], mm_dt, name=f"wh{l}")
                nc.sync.dma_start(out=w_l, in_=wh[l, :, :])
                wh_sb.append(w_l)
            wo_sb = wpool.tile([128, 3 * N_STREAMS], mm_dt)
            nc.sync.dma_start(out=wo_sb, in_=wo[:, :])

            def mm_layer(ps, w_sb, rhs, m=None):
                for k in range(blocks):
                    bs = slice(k * mm_block, (k + 1) * mm_block)
                    o = ps[:, bs] if m is None else ps[:m, bs]
                    nc.tensor.matmul(o, w_sb, rhs[:, bs], start=True, stop=True)

            for p in range(n_grp):
                css = [slice((GRP * p + i) * chunk, (GRP * p + i + 1) * chunk)
                       for i in range(GRP)]
                x_sb, h = [None] * GRP, [None] * GRP
                ps = [None] * GRP
                for i in range(GRP):
                    x_sb[i] = xpool.tile([2 * N_STREAMS, chunk], mm_dt, tag="x",
                                         name=f"x{i}")
                    nc.sync.dma_start(out=x_sb[i], in_=xT[:, css[i]])

                # layer 0
                for i in range(GRP):
                    ps[i] = psum_pool.tile([128, chunk], f32, tag="ps", name=f"ps{i}")
                    mm_layer(ps[i], w0_sb, x_sb[i])
                    h[i] = hpool.tile([128, chunk], mm_dt, tag="h", name=f"h{i}")
                    nc.scalar.activation(h[i], ps[i], AF.Tanh, bias=b0_sb)

                # hidden layers
                for l in range(N_HID_LAYERS):
                    for i in range(GRP):
                        ps[i] = psum_pool.tile([128, chunk], f32, tag="ps",
                                               name=f"ps{i}")
                        mm_layer(ps[i], wh_sb[l], h[i])
                        h_next = hpool.tile([128, chunk], mm_dt, tag="h",
                                            name=f"h{i}")
                        nc.scalar.activation(h_next, ps[i], AF.Tanh)
                        h[i] = h_next

                # output layer: stack the 4 col-blocks of each chunk onto 4
                # PSUM partition groups (32k..32k+24) so sigmoid runs dense
                # on [128, 2048] once per quad instead of 4x [24, 2048].
                ps_o = psum_pool.tile([128, chunk], f32, tag="ps", name="pso")
                for i in range(GRP):
                    for k in range(blocks):
                        bs = slice(k * mm_block, (k + 1) * mm_block)
                        os_ = slice(i * mm_block, (i + 1) * mm_block)
                        nc.tensor.matmul(
                            ps_o[32 * k:32 * k + 3 * N_STREAMS, os_], wo_sb,
                            h[i][:, bs], start=True, stop=True,
                            tile_position=(0, 32 * k))
                o_grp = opool.tile([128, GRP * mm_block], f32, tag="o", name="o")
                nc.scalar.activation(o_grp, ps_o[:, :GRP * mm_block], AF.Sigmoid)
                # o_grp[32k+m, 512i+c] = out[m, (GRP*p+i)*2048 + 512k + c]
                for k in range(blocks):
                    src = o_grp[32 * k:32 * k + 3 * N_STREAMS, :]
                    dst = out.rearrange("m (q i kk c) -> kk m q i c",
                                        q=n_grp, i=GRP, kk=blocks)[k, :, p]
                    nc.sync.dma_start(out=dst, in_=src)

    nc.compile()
    _GRAPH_CACHE[key] = nc
    return nc


def _pack_weights(W0, b0, Wh, Wout):
    w0p = np.zeros((2 * N_STREAMS, 128), np.float32)
    b0p = np.zeros((128, 1), np.float32)
    whp = np.zeros((N_HID_LAYERS, 128, 128), np.float32)
    wop = np.zeros((128, 3 * N_STREAMS), np.float32)
    for s in range(N_STREAMS):
        w0p[2 * s:2 * s + 2, 16 * s:16 * s + 16] = W0.T
        b0p[16 * s:16 * s + 16, 0] = b0
        for l in range(N_HID_LAYERS):
            whp[l, 16 * s:16 * s + 16, 16 * s:16 * s + 16] = Wh[l].T
        wop[16 * s:16 * s + 16, 3 * s:3 * s + 3] = Wout.T
    return w0p, b0p, whp, wop


def kernel(x, W0, b0, Wh, Wout, trace=False):
    x = np.asarray(x, np.float32)
    W0 = np.asarray(W0, np.float32)
    b0 = np.asarray(b0, np.float32)
    Wh = np.asarray(Wh, np.float32)
    Wout = np.asarray(Wout, np.float32)

    nc = _build_graph(N_COLS)
    w0p, b0p, whp, wop = _pack_weights(W0, b0, Wh, Wout)
    w0p, whp, wop = (a.astype(BF16) for a in (w0p, whp, wop))

    per_core = N_STREAMS * N_COLS
    in_maps = []
    for c in range(N_CORES):
        xs = x[c * per_core:(c + 1) * per_core]
        xT = np.ascontiguousarray(
            xs.reshape(N_STREAMS, N_COLS, 2).transpose(0, 2, 1)
        ).reshape(2 * N_STREAMS, N_COLS)
        in_maps.append({"xT": xT.astype(BF16), "w0": w0p, "b0": b0p,
                        "wh": whp, "wo": wop})

    res = run_bass_kernel_spmd(nc, in_maps, core_ids=list(range(N_CORES)),
                               trace=trace)
    LAST_RUN_INFO.clear()
    LAST_RUN_INFO["exec_time_ns"] = res.exec_time_ns
    prof = getattr(res, "instructions_and_trace", None)
    LAST_RUN_INFO["trace_dir"] = getattr(prof, "trace_dir", None)

    parts = []
    for r in res.results:
        o = r["out"].reshape(N_STREAMS, 3, N_COLS).transpose(0, 2, 1)
        parts.append(o.reshape(per_core, 3))
    return np.concatenate(parts, axis=0)
